# revision 32
# baseline (speedup 1.0000x reference)
"""Trainium2 Bass kernel for the (non-standard) MultiHeadAttention module.

Reference math (B=4, N=2048, E=512, H=8):
    q/k/v  = x @ W{q,k,v} + b          # (B, N, E*H)
    split:   head h takes columns h::H  -> per-head (N, E) matrices
    attT_h = (k_h^T @ q_h) * 1/sqrt(N) # (f, e) -- attention over the E axis
    A_h    = exp(attT_h)               # softmax numerator (no max-sub
                                       #  needed, logits are O(+-5))
    s_h[e] = sum_f A_h[f, e]
    out row n' = 4e + r gets  sum_hl (A_h^T/s_h) @ P_h + bp
      for h = 2r + hl  (consequence of the reference's raw
      (B,E,H,N)->(B,N,E*H) reshape before the output projection), where
    P_h    = v_h^T @ Wp_half(hl) + bp/2

Key algebraic refactors (this module attends over the E axis and contracts
over n, so everything collapses into E x E space):
  * Gram matrix  X = x_b^T @ x_b  (E x E, once per core):
      attT_h = Wk_h^T X Wq_h + (Wk_h^T xs) (x) bq_h
               + bk_h (x) (Wq_h^T xs + N bq_h),   xs = colsum(x_b)
    -- eliminates the q/k projections entirely.
  * (A @ v^T) @ Wp == A @ (v^T @ Wp) and
    v_h^T @ Wp_hl == Wv_h^T @ G_hl + bv_h (x) colsum(Wp_hl)  with
    G_hl = x_b^T @ Wp_hl computed once per core -- eliminates the v
    projection and the big P matmuls.
  * bp/2 folded into each P_h; softmax normalization at the very end:
    out = U0*r0 + U1*r1,  U_h = A_h^T @ P_h,  r_h = 1/s_h.
  Net: ~7.5 GFLOP and ~560 matmuls per core vs ~26 GFLOP naively.

Sharding: 16 independent units (b, r), b in 0..3, r in 0..3; unit (b, r)
owns heads {2r, 2r+1} and produces output rows out[b, r::4, :].  Two units
per core, batch-major:  core c -> b = c//2, r in {2*(c%2), 2*(c%2)+1}.
No inter-core communication.

All matmuls run as float32r (fp32 storage, reduced-precision single-pass
PE mode: full speed for moving-free-dim >= 256).
"""

import numpy as np
from contextlib import ExitStack

import concourse.bass as bass
import concourse.mybir as mybir
import concourse.tile as tile
from concourse import bacc
from concourse.bass_utils import run_bass_kernel_spmd

B, N, E, H = 4, 2048, 512, 8
NT = N // 128          # 16 contraction chunks of 128 over n
EB = E // 128          # 4 blocks of 128 over e/f
SCALE = float(1.0 / np.sqrt(np.float32(N)))
F32 = mybir.dt.float32
F32R = mybir.dt.float32r
PSUM = bass.MemorySpace.PSUM

_CACHED_NC = None


def _bcast128(ap_nd):
    """DMA access pattern replicating a DRAM region across 128 partitions."""
    return bass.AP(
        tensor=ap_nd.tensor, offset=ap_nd.offset, ap=[[0, 128]] + list(ap_nd.ap)
    )


def build_nc():
    nc = bacc.Bacc("TRN2", target_bir_lowering=False, debug=False)

    xn_d = nc.dram_tensor("xn", (N, E), F32R, kind="ExternalInput")
    wq_d = nc.dram_tensor("wq", (2, 2, 128, EB, E), F32R, kind="ExternalInput")
    wk_d = nc.dram_tensor("wk", (2, 2, 128, EB, E), F32R, kind="ExternalInput")
    wv_d = nc.dram_tensor("wv", (2, 2, 128, EB, E), F32R, kind="ExternalInput")
    wp_d = nc.dram_tensor("wp", (2, N, E), F32R, kind="ExternalInput")
    swp_d = nc.dram_tensor("swp", (1, 2, E), F32R, kind="ExternalInput")
    hvec_d = nc.dram_tensor("hvec", (1, 2, 2, 2, E), F32R, kind="ExternalInput")
    bqkv_d = nc.dram_tensor("bqkv", (2, 2, 1, 3, E), F32R, kind="ExternalInput")
    bph_d = nc.dram_tensor("bph", (E,), F32, kind="ExternalInput")
    ones_d = nc.dram_tensor("ones", (128, 2), F32R, kind="ExternalInput")
    out_d = nc.dram_tensor("out", (2, E, E), F32, kind="ExternalOutput")

    with tile.TileContext(nc) as tc, ExitStack() as ctx:
        consts = ctx.enter_context(tc.tile_pool(name="consts", bufs=1))
        stream = ctx.enter_context(tc.tile_pool(name="stream", bufs=4))
        wqkv_pool = ctx.enter_context(tc.tile_pool(name="wqkv", bufs=2))
        bias_pool = ctx.enter_context(tc.tile_pool(name="bias", bufs=2))
        t1_pool = ctx.enter_context(tc.tile_pool(name="t1", bufs=1))
        a_pool = ctx.enter_context(tc.tile_pool(name="a", bufs=2))
        p_pool = ctx.enter_context(tc.tile_pool(name="p", bufs=2))
        o_pool = ctx.enter_context(tc.tile_pool(name="o", bufs=4))
        r_pool = ctx.enter_context(tc.tile_pool(name="r", bufs=2))
        mm_ps = ctx.enter_context(tc.tile_pool(name="mmps", bufs=2, space=PSUM))
        big_ps = ctx.enter_context(tc.tile_pool(name="bigps", bufs=1, space=PSUM))
        u_ps = ctx.enter_context(tc.tile_pool(name="ups", bufs=2, space=PSUM))

        # Prime the G-phase wp streams (first chunks of each half) before
        # anything else on their queues.
        wp_primed = {}
        for pn in range(3):
            psl = slice(pn * 128, (pn + 1) * 128)
            w0 = stream.tile([128, E], F32R, tag="wp0", name=f"wp0p{pn}")
            nc.gpsimd.dma_start(out=w0[:], in_=wp_d.ap()[0, psl, :])
            w1 = stream.tile([128, E], F32R, tag="wp1", name=f"wp1p{pn}")
            nc.scalar.dma_start(out=w1[:], in_=wp_d.ap()[1, psl, :])
            wp_primed[pn] = (w0, w1)

        # x (natural layout), resident: feeds both the X and G phases.
        # The X phase only gates on these.
        xn_sb = []
        for n in range(NT):
            t = consts.tile([128, E], F32R, tag=f"xn{n}", name=f"xn{n}")
            nc.sync.dma_start(out=t[:], in_=xn_d.ap()[n * 128 : (n + 1) * 128, :])
            xn_sb.append(t)

        # ---- other resident constants ----
        ones_sb = consts.tile([128, 2], F32R, tag="ones")
        nc.gpsimd.dma_start(out=ones_sb[:], in_=ones_d.ap())
        bph_sb = consts.tile([128, E], F32, tag="bph")
        nc.scalar.dma_start(out=bph_sb[:], in_=_bcast128(bph_d.ap()))
        swp_sb = consts.tile([1, 2, E], F32R, tag="swp")
        nc.gpsimd.dma_start(out=swp_sb[:], in_=swp_d.ap())
        hvec_sb = consts.tile([1, 2, 2, 2, E], F32R, tag="hvec")
        nc.scalar.dma_start(out=hvec_sb[:], in_=hvec_d.ap())

        # ---- pass 1: X = x^T x (big: 4 banks) + G0 = x^T Wp0 (mm+u: 4
        # banks), one shared sweep over n so X is not xn-starved alone ----
        X_ps = big_ps.tile([128, EB, E], F32, tag="big")
        g_sb = [
            consts.tile([128, EB, E], F32R, tag=f"g{hl}", name=f"g{hl}")
            for hl in range(2)
        ]
        g0_slots = [
            mm_ps.tile([128, E], F32, tag="mm", name="g0a"),
            mm_ps.tile([128, E], F32, tag="mm", name="g0b"),
            u_ps.tile([128, E], F32, tag="u", name="g0c"),
            u_ps.tile([128, E], F32, tag="u", name="g0d"),
        ]
        gate_gearly = None
        for n in range(NT):
            nsl = slice(n * 128, (n + 1) * 128)
            if n in wp_primed:
                wp0_sb, _ = wp_primed[n]
            else:
                wp0_sb = stream.tile([128, E], F32R, tag="wp0")
                nc.gpsimd.dma_start(out=wp0_sb[:], in_=wp_d.ap()[0, nsl, :])
            for m in range(EB):
                msl = slice(m * 128, (m + 1) * 128)
                nc.tensor.matmul(
                    X_ps[:, m, :],
                    xn_sb[n][:, msl],
                    xn_sb[n][:],
                    start=n == 0,
                    stop=n == NT - 1,
                )
                g_bi = nc.tensor.matmul(
                    g0_slots[m][:],
                    xn_sb[n][:, msl],
                    wp0_sb[:],
                    start=n == 0,
                    stop=n == NT - 1,
                )
                if n == 2 and m == 0:
                    gate_gearly = g_bi.ins
        X_sb = consts.tile([128, EB, E], F32R, tag="X")
        for m in range(EB):
            nc.vector.tensor_copy(X_sb[:, m, :], X_ps[:, m, :])
            nc.vector.tensor_copy(g_sb[0][:, m, :], g0_slots[m][:])

        # ---- pass 2: G1 = x^T Wp1 (mm+u slots again) ----
        g1_slots = [
            mm_ps.tile([128, E], F32, tag="mm", name="g1a"),
            mm_ps.tile([128, E], F32, tag="mm", name="g1b"),
            u_ps.tile([128, E], F32, tag="u", name="g1c"),
            u_ps.tile([128, E], F32, tag="u", name="g1d"),
        ]
        gate_gmid = None
        for n in range(NT):
            nsl = slice(n * 128, (n + 1) * 128)
            if n in wp_primed:
                _, wp1_sb = wp_primed[n]
            else:
                wp1_sb = stream.tile([128, E], F32R, tag="wp1")
                nc.scalar.dma_start(out=wp1_sb[:], in_=wp_d.ap()[1, nsl, :])
            for m in range(EB):
                msl = slice(m * 128, (m + 1) * 128)
                g_bi = nc.tensor.matmul(
                    g1_slots[m][:],
                    xn_sb[n][:, msl],
                    wp1_sb[:],
                    start=n == 0,
                    stop=n == NT - 1,
                )
                if n == NT // 2 and m == 0:
                    gate_gmid = g_bi.ins
        for m in range(EB):
            nc.vector.tensor_copy(g_sb[1][:, m, :], g1_slots[m][:])

        gate_hist = [gate_gearly, gate_gearly]  # per-head early gates
        pending_s = None

        def emit_pending_s():
            nonlocal pending_s
            if pending_s is None:
                return
            A_sb, R_list = pending_s
            pending_s = None
            s_ps = mm_ps.tile([128, EB, 2], F32, tag="mm")
            for eb in range(EB):
                esl = slice(eb * 128, (eb + 1) * 128)
                for fc in range(EB):
                    nc.tensor.matmul(
                        s_ps[:, eb, :],
                        A_sb[:, fc, esl],
                        ones_sb[:],
                        start=fc == 0,
                        stop=fc == EB - 1,
                    )
            r_sb = r_pool.tile([128, EB, 2], F32, tag="r")
            nc.vector.reciprocal(out=r_sb[:], in_=s_ps[:])
            R_list.append(r_sb)

        for u in range(2):
            A_tiles, P_tiles, R_tiles = [], [], []
            for hl in range(2):
                # --- weights + biases for head (u, hl), prefetch-gated ---
                bias_sb = bias_pool.tile([1, 3, E], F32R, tag="bias")
                bias_bi = nc.scalar.dma_start(
                    out=bias_sb[:], in_=bqkv_d.ap()[u, hl]
                )
                wv_sb = wqkv_pool.tile([128, EB, E], F32R, tag="wv")
                wv_bi = nc.gpsimd.dma_start(out=wv_sb[:], in_=wv_d.ap()[u, hl])
                wq_sb = wqkv_pool.tile([128, EB, E], F32R, tag="wq")
                wq_bi = nc.gpsimd.dma_start(out=wq_sb[:], in_=wq_d.ap()[u, hl])
                wk_sb = wqkv_pool.tile([128, EB, E], F32R, tag="wk")
                wk_bi = nc.scalar.dma_start(out=wk_sb[:], in_=wk_d.ap()[u, hl])
                gate = gate_hist[-2]  # two head-phases back
                for bi in (bias_bi, wv_bi, wq_bi, wk_bi):
                    tile.add_dep_helper(bi.ins, gate, reason="delay prefetch")

                # --- P_h = Wv_h^T @ G_hl + bv_h (x) swp_hl + bp/2 ---
                # (independent of the attention path; fills the PE while the
                #  previous head's exp runs on ACT)
                P_sb = p_pool.tile([128, EB, E], F32R, tag="p")

                def emit_p_group(fb):
                    fsl = slice(fb * 128, (fb + 1) * 128)
                    p_ps = u_ps.tile([128, E], F32, tag="u", name=f"pp{fb}")
                    first = None
                    for ec in range(EB):
                        bi = nc.tensor.matmul(
                            p_ps[:],
                            wv_sb[:, ec, fsl],
                            g_sb[hl][:, ec, :],
                            start=ec == 0,
                            stop=False,
                        )
                        first = first or bi
                    nc.tensor.matmul(
                        p_ps[:],
                        bias_sb[0:1, 2, fsl],
                        swp_sb[0:1, hl, :],
                        start=False,
                        stop=True,
                    )
                    nc.vector.tensor_add(P_sb[:, fb, :], p_ps[:], bph_sb[:])
                    return first

                # first half of P (covers the previous head's exp wait)
                p_first = emit_p_group(0)
                emit_p_group(1)
                gate_early = p_first.ins

                # --- T1 = X @ Wq_h (uses X symmetry: lhsT = X slices) ---
                T1_ps = big_ps.tile([128, EB, E], F32, tag="big")
                t1_first = None
                for m in range(EB):
                    msl = slice(m * 128, (m + 1) * 128)
                    for ec in range(EB):
                        bi = nc.tensor.matmul(
                            T1_ps[:, m, :],
                            X_sb[:, ec, msl],
                            wq_sb[:, ec, :],
                            start=ec == 0,
                            stop=ec == EB - 1,
                        )
                        t1_first = t1_first or bi
                gate_mid = t1_first.ins
                T1_sb = t1_pool.tile([128, EB, E], F32R, tag="t1")
                for m in range(EB):
                    nc.vector.tensor_copy(T1_sb[:, m, :], T1_ps[:, m, :])

                # second half of P + previous head's s: both independent of
                # T1, they cover the T1 psum->sbuf copy before attT
                emit_p_group(2)
                emit_p_group(3)
                P_tiles.append(P_sb)
                emit_pending_s()

                # --- attT = Wk_h^T @ T1 + uvec (x) bq + bk (x) wvec ---
                attT_ps = big_ps.tile([128, EB, E], F32, tag="big")
                for fb in range(EB):
                    fsl = slice(fb * 128, (fb + 1) * 128)
                    for ec in range(EB):
                        nc.tensor.matmul(
                            attT_ps[:, fb, :],
                            wk_sb[:, ec, fsl],
                            T1_sb[:, ec, :],
                            start=ec == 0,
                            stop=False,
                        )
                    nc.tensor.matmul(
                        attT_ps[:, fb, :],
                        hvec_sb[0:1, u, hl, 0, fsl],
                        bias_sb[0:1, 0, :],
                        start=False,
                        stop=False,
                    )
                    nc.tensor.matmul(
                        attT_ps[:, fb, :],
                        bias_sb[0:1, 1, fsl],
                        hvec_sb[0:1, u, hl, 1, :],
                        start=False,
                        stop=True,
                    )

                # --- exp (softmax numerator, transposed layout) ---
                A_sb = a_pool.tile([128, EB, E], F32R, tag="a")
                for fb in range(EB):
                    nc.scalar.activation(
                        out=A_sb[:, fb, :],
                        in_=attT_ps[:, fb, :],
                        func=mybir.ActivationFunctionType.Exp,
                        scale=SCALE,
                    )
                A_tiles.append(A_sb)
                pending_s = (A_sb, R_tiles)
                gate_hist.append(gate_early)

            # --- U_h = A_h^T @ P_h ; out = U0*r0 + U1*r1 ---
            out_tiles = [
                o_pool.tile([128, E], F32, tag="o", name=f"ot{u}_{i}")
                for i in range(EB)
            ]
            for hl in range(2):
                if hl == 1:
                    emit_pending_s()  # s of this unit's second head
                for eb in range(EB):
                    esl = slice(eb * 128, (eb + 1) * 128)
                    u_tile = u_ps.tile([128, E], F32, tag="u")
                    for fc in range(EB):
                        nc.tensor.matmul(
                            u_tile[:],
                            A_tiles[hl][:, fc, esl],
                            P_tiles[hl][:, fc, :],
                            start=fc == 0,
                            stop=fc == EB - 1,
                        )
                    if hl == 0:
                        nc.vector.tensor_scalar_mul(
                            out_tiles[eb][:], u_tile[:], R_tiles[0][:, eb, 0:1]
                        )
                    else:
                        nc.vector.scalar_tensor_tensor(
                            out_tiles[eb][:],
                            u_tile[:],
                            R_tiles[1][:, eb, 0:1],
                            out_tiles[eb][:],
                            op0=mybir.AluOpType.mult,
                            op1=mybir.AluOpType.add,
                        )
                    if hl == 1:
                        nc.sync.dma_start(
                            out=out_d.ap()[u, eb * 128 : (eb + 1) * 128, :],
                            in_=out_tiles[eb][:],
                        )

    nc.compile()
    return nc


def _get_nc():
    global _CACHED_NC
    if _CACHED_NC is None:
        _CACHED_NC = build_nc()
    return _CACHED_NC


def make_in_maps(x, Wq, bq, Wk, bk, Wv, bv, Wp, bp):
    x = np.asarray(x, np.float32)
    Wq, Wk, Wv, Wp = (np.asarray(a, np.float32) for a in (Wq, Wk, Wv, Wp))
    bq, bk, bv, bp = (np.asarray(a, np.float32) for a in (bq, bk, bv, bp))
    wp_arr = np.ascontiguousarray(np.stack([Wp[:N], Wp[N:]]))
    swp = np.ascontiguousarray(np.stack([Wp[:N].sum(0), Wp[N:].sum(0)])[None])
    bph = np.ascontiguousarray(0.5 * bp)
    in_maps = []
    for c in range(8):
        b = c // 2
        rs = [2 * (c % 2), 2 * (c % 2) + 1]
        heads = [[2 * r + hl for hl in range(2)] for r in rs]
        xs = x[b].sum(0)

        def tile_w(Wm, h):
            # (E, E) -> [p, t, e] with row t*128+p on partition p
            return Wm[:, h::H].reshape(EB, 128, E).transpose(1, 0, 2)

        wq_arr = np.ascontiguousarray(
            np.stack([[tile_w(Wq, h) for h in hu] for hu in heads])
        )
        wk_arr = np.ascontiguousarray(
            np.stack([[tile_w(Wk, h) for h in hu] for hu in heads])
        )
        wv_arr = np.ascontiguousarray(
            np.stack([[tile_w(Wv, h) for h in hu] for hu in heads])
        )
        bqkv = np.ascontiguousarray(
            np.stack([[[bq[h::H], bk[h::H], bv[h::H]] for h in hu] for hu in heads])[
                :, :, None
            ]
        )
        hvec = np.ascontiguousarray(
            np.stack(
                [
                    [
                        [
                            Wk[:, h::H].T @ xs,
                            Wq[:, h::H].T @ xs + np.float32(N) * bq[h::H],
                        ]
                        for h in hu
                    ]
                    for hu in heads
                ]
            )[None]
        )
        in_maps.append(
            {
                "xn": np.ascontiguousarray(x[b]),
                "wq": wq_arr,
                "wk": wk_arr,
                "wv": wv_arr,
                "wp": wp_arr,
                "swp": swp,
                "hvec": hvec,
                "bqkv": bqkv,
                "bph": bph,
                "ones": np.ones((128, 2), np.float32),
            }
        )
    return in_maps


def assemble_out(results):
    out = np.empty((B, N, E), np.float32)
    for c in range(8):
        b = c // 2
        for ui in range(2):
            r = 2 * (c % 2) + ui
            out[b, r::4, :] = results[c]["out"][ui]
    return out


def run(inputs, trace=False, **spmd_kwargs):
    """Full pipeline; returns (output, BassKernelResults)."""
    nc = _get_nc()
    in_maps = make_in_maps(**inputs)
    res = run_bass_kernel_spmd(
        nc, in_maps, core_ids=list(range(8)), trace=trace, **spmd_kwargs
    )
    return assemble_out(res.results), res


def kernel(**inputs):
    out, _ = run(inputs)
    return out


# revision 33
# speedup vs baseline: 1.0426x; 1.0426x over previous
"""Trainium2 Bass kernel for the (non-standard) MultiHeadAttention module.

Reference math (B=4, N=2048, E=512, H=8):
    q/k/v  = x @ W{q,k,v} + b          # (B, N, E*H)
    split:   head h takes columns h::H  -> per-head (N, E) matrices
    attT_h = (k_h^T @ q_h) * 1/sqrt(N) # (f, e) -- attention over the E axis
    A_h    = exp(attT_h)               # softmax numerator (no max-sub
                                       #  needed, logits are O(+-5))
    s_h[e] = sum_f A_h[f, e]
    out row n' = 4e + r gets  sum_hl (A_h^T/s_h) @ P_h + bp
      for h = 2r + hl  (consequence of the reference's raw
      (B,E,H,N)->(B,N,E*H) reshape before the output projection), where
    P_h    = v_h^T @ Wp_half(hl) + bp/2

Key algebraic refactors (this module attends over the E axis and contracts
over n, so everything collapses into E x E space):
  * Gram matrix  X = x_b^T @ x_b  (E x E, once per core):
      attT_h = Wk_h^T X Wq_h + (Wk_h^T xs) (x) bq_h
               + bk_h (x) (Wq_h^T xs + N bq_h),   xs = colsum(x_b)
    -- eliminates the q/k projections entirely.
  * (A @ v^T) @ Wp == A @ (v^T @ Wp) and
    v_h^T @ Wp_hl == Wv_h^T @ G_hl + bv_h (x) colsum(Wp_hl)  with
    G_hl = x_b^T @ Wp_hl computed once per core -- eliminates the v
    projection and the big P matmuls.
  * bp/2 folded into each P_h; softmax normalization at the very end:
    out = U0*r0 + U1*r1,  U_h = A_h^T @ P_h,  r_h = 1/s_h.
  Net: ~7.5 GFLOP and ~560 matmuls per core vs ~26 GFLOP naively.

Sharding: 16 independent units (b, r), b in 0..3, r in 0..3; unit (b, r)
owns heads {2r, 2r+1} and produces output rows out[b, r::4, :].  Two units
per core, batch-major:  core c -> b = c//2, r in {2*(c%2), 2*(c%2)+1}.
No inter-core communication.

All matmuls run as float32r (fp32 storage, reduced-precision single-pass
PE mode: full speed for moving-free-dim >= 256).
"""

import numpy as np
from contextlib import ExitStack

import concourse.bass as bass
import concourse.mybir as mybir
import concourse.tile as tile
from concourse import bacc
from concourse.bass_utils import run_bass_kernel_spmd

B, N, E, H = 4, 2048, 512, 8
NT = N // 128          # 16 contraction chunks of 128 over n
EB = E // 128          # 4 blocks of 128 over e/f
SCALE = float(1.0 / np.sqrt(np.float32(N)))
F32 = mybir.dt.float32
F32R = mybir.dt.float32r
PSUM = bass.MemorySpace.PSUM

_CACHED_NC = None


def _bcast128(ap_nd):
    """DMA access pattern replicating a DRAM region across 128 partitions."""
    return bass.AP(
        tensor=ap_nd.tensor, offset=ap_nd.offset, ap=[[0, 128]] + list(ap_nd.ap)
    )


def build_nc():
    nc = bacc.Bacc("TRN2", target_bir_lowering=False, debug=False)

    xn_d = nc.dram_tensor("xn", (N, E), F32R, kind="ExternalInput")
    wq_d = nc.dram_tensor("wq", (2, 2, 128, EB, E), F32R, kind="ExternalInput")
    wk_d = nc.dram_tensor("wk", (2, 2, 128, EB, E), F32R, kind="ExternalInput")
    wv_d = nc.dram_tensor("wv", (2, 2, 128, EB, E), F32R, kind="ExternalInput")
    wp_d = nc.dram_tensor("wp", (2, N, E), F32R, kind="ExternalInput")
    swp_d = nc.dram_tensor("swp", (1, 2, E), F32R, kind="ExternalInput")
    hvec_d = nc.dram_tensor("hvec", (1, 2, 2, 2, E), F32R, kind="ExternalInput")
    bqkv_d = nc.dram_tensor("bqkv", (2, 2, 1, 3, E), F32R, kind="ExternalInput")
    bph_d = nc.dram_tensor("bph", (E,), F32, kind="ExternalInput")
    ones_d = nc.dram_tensor("ones", (128, 2), F32R, kind="ExternalInput")
    out_d = nc.dram_tensor("out", (2, E, E), F32, kind="ExternalOutput")

    with tile.TileContext(nc) as tc, ExitStack() as ctx:
        consts = ctx.enter_context(tc.tile_pool(name="consts", bufs=1))
        stream = ctx.enter_context(tc.tile_pool(name="stream", bufs=4))
        wqkv_pool = ctx.enter_context(tc.tile_pool(name="wqkv", bufs=2))
        bias_pool = ctx.enter_context(tc.tile_pool(name="bias", bufs=2))
        t1_pool = ctx.enter_context(tc.tile_pool(name="t1", bufs=1))
        a_pool = ctx.enter_context(tc.tile_pool(name="a", bufs=2))
        p_pool = ctx.enter_context(tc.tile_pool(name="p", bufs=2))
        o_pool = ctx.enter_context(tc.tile_pool(name="o", bufs=4))
        r_pool = ctx.enter_context(tc.tile_pool(name="r", bufs=2))
        mm_ps = ctx.enter_context(tc.tile_pool(name="mmps", bufs=2, space=PSUM))
        big_ps = ctx.enter_context(tc.tile_pool(name="bigps", bufs=1, space=PSUM))
        u_ps = ctx.enter_context(tc.tile_pool(name="ups", bufs=2, space=PSUM))

        # Prime the G-phase wp streams (first chunks of each half) before
        # anything else on their queues.
        wp_primed = {}
        for pn in range(3):
            psl = slice(pn * 128, (pn + 1) * 128)
            w0 = stream.tile([128, E], F32R, tag="wp0", name=f"wp0p{pn}")
            nc.gpsimd.dma_start(out=w0[:], in_=wp_d.ap()[0, psl, :])
            w1 = stream.tile([128, E], F32R, tag="wp1", name=f"wp1p{pn}", bufs=6)
            nc.scalar.dma_start(out=w1[:], in_=wp_d.ap()[1, psl, :])
            wp_primed[pn] = (w0, w1)

        # x (natural layout), resident: feeds both the X and G phases.
        # The X phase only gates on these.
        xn_sb = []
        for n in range(NT):
            t = consts.tile([128, E], F32R, tag=f"xn{n}", name=f"xn{n}")
            eng = nc.sync if n % 2 == 0 else nc.scalar
            eng.dma_start(out=t[:], in_=xn_d.ap()[n * 128 : (n + 1) * 128, :])
            xn_sb.append(t)

        # ---- other resident constants ----
        ones_sb = consts.tile([128, 2], F32R, tag="ones")
        nc.gpsimd.dma_start(out=ones_sb[:], in_=ones_d.ap())
        bph_sb = consts.tile([128, E], F32, tag="bph")
        nc.scalar.dma_start(out=bph_sb[:], in_=_bcast128(bph_d.ap()))
        swp_sb = consts.tile([1, 2, E], F32R, tag="swp")
        nc.gpsimd.dma_start(out=swp_sb[:], in_=swp_d.ap())
        hvec_sb = consts.tile([1, 2, 2, 2, E], F32R, tag="hvec")
        nc.scalar.dma_start(out=hvec_sb[:], in_=hvec_d.ap())

        # ---- pass 1: X = x^T x (big: 4 banks) + G0 = x^T Wp0 (mm+u: 4
        # banks), one shared sweep over n so X is not xn-starved alone ----
        X_ps = big_ps.tile([128, EB, E], F32, tag="big")
        g_sb = [
            consts.tile([128, EB, E], F32R, tag=f"g{hl}", name=f"g{hl}")
            for hl in range(2)
        ]
        g0_slots = [
            mm_ps.tile([128, E], F32, tag="mm", name="g0a"),
            mm_ps.tile([128, E], F32, tag="mm", name="g0b"),
            u_ps.tile([128, E], F32, tag="u", name="g0c"),
            u_ps.tile([128, E], F32, tag="u", name="g0d"),
        ]
        gate_gearly = None
        for n in range(NT):
            nsl = slice(n * 128, (n + 1) * 128)
            if n in wp_primed:
                wp0_sb, _ = wp_primed[n]
            else:
                wp0_sb = stream.tile([128, E], F32R, tag="wp0")
                nc.gpsimd.dma_start(out=wp0_sb[:], in_=wp_d.ap()[0, nsl, :])
            for m in range(EB):
                msl = slice(m * 128, (m + 1) * 128)
                nc.tensor.matmul(
                    X_ps[:, m, :],
                    xn_sb[n][:, msl],
                    xn_sb[n][:],
                    start=n == 0,
                    stop=n == NT - 1,
                )
                g_bi = nc.tensor.matmul(
                    g0_slots[m][:],
                    xn_sb[n][:, msl],
                    wp0_sb[:],
                    start=n == 0,
                    stop=n == NT - 1,
                )
                if n == 2 and m == 0:
                    gate_gearly = g_bi.ins
        X_sb = consts.tile([128, EB, E], F32R, tag="X")
        for m in range(EB):
            nc.vector.tensor_copy(X_sb[:, m, :], X_ps[:, m, :])
            nc.vector.tensor_copy(g_sb[0][:, m, :], g0_slots[m][:])

        # ---- pass 2: G1 = x^T Wp1 (mm+u slots again) ----
        g1_slots = [
            mm_ps.tile([128, E], F32, tag="mm", name="g1a"),
            mm_ps.tile([128, E], F32, tag="mm", name="g1b"),
            u_ps.tile([128, E], F32, tag="u", name="g1c"),
            u_ps.tile([128, E], F32, tag="u", name="g1d"),
        ]
        gate_gmid = None
        for n in range(NT):
            nsl = slice(n * 128, (n + 1) * 128)
            if n in wp_primed:
                _, wp1_sb = wp_primed[n]
            else:
                wp1_sb = stream.tile([128, E], F32R, tag="wp1", bufs=6)
                nc.scalar.dma_start(out=wp1_sb[:], in_=wp_d.ap()[1, nsl, :])
            for m in range(EB):
                msl = slice(m * 128, (m + 1) * 128)
                g_bi = nc.tensor.matmul(
                    g1_slots[m][:],
                    xn_sb[n][:, msl],
                    wp1_sb[:],
                    start=n == 0,
                    stop=n == NT - 1,
                )
                if n == NT // 2 and m == 0:
                    gate_gmid = g_bi.ins
        for m in range(EB):
            nc.vector.tensor_copy(g_sb[1][:, m, :], g1_slots[m][:])

        gate_hist = [gate_gearly, gate_gearly]  # per-head early gates
        pending_s = None

        def emit_pending_s():
            nonlocal pending_s
            if pending_s is None:
                return
            A_sb, R_list = pending_s
            pending_s = None
            s_ps = mm_ps.tile([128, EB, 2], F32, tag="mm")
            for eb in range(EB):
                esl = slice(eb * 128, (eb + 1) * 128)
                for fc in range(EB):
                    nc.tensor.matmul(
                        s_ps[:, eb, :],
                        A_sb[:, fc, esl],
                        ones_sb[:],
                        start=fc == 0,
                        stop=fc == EB - 1,
                    )
            r_sb = r_pool.tile([128, EB, 2], F32, tag="r")
            nc.vector.reciprocal(out=r_sb[:], in_=s_ps[:])
            R_list.append(r_sb)

        for u in range(2):
            A_tiles, P_tiles, R_tiles = [], [], []
            for hl in range(2):
                # --- weights + biases for head (u, hl), prefetch-gated ---
                bias_sb = bias_pool.tile([1, 3, E], F32R, tag="bias")
                bias_bi = nc.scalar.dma_start(
                    out=bias_sb[:], in_=bqkv_d.ap()[u, hl]
                )
                wv_sb = wqkv_pool.tile([128, EB, E], F32R, tag="wv")
                wv_bi = nc.gpsimd.dma_start(out=wv_sb[:], in_=wv_d.ap()[u, hl])
                wq_sb = wqkv_pool.tile([128, EB, E], F32R, tag="wq")
                wq_bi = nc.gpsimd.dma_start(out=wq_sb[:], in_=wq_d.ap()[u, hl])
                wk_sb = wqkv_pool.tile([128, EB, E], F32R, tag="wk")
                wk_bi = nc.scalar.dma_start(out=wk_sb[:], in_=wk_d.ap()[u, hl])
                gate = gate_hist[-2]  # two head-phases back
                for bi in (bias_bi, wv_bi, wq_bi, wk_bi):
                    tile.add_dep_helper(bi.ins, gate, reason="delay prefetch")

                # --- P_h = Wv_h^T @ G_hl + bv_h (x) swp_hl + bp/2 ---
                # (independent of the attention path; fills the PE while the
                #  previous head's exp runs on ACT)
                P_sb = p_pool.tile([128, EB, E], F32R, tag="p")

                def emit_p_group(fb):
                    fsl = slice(fb * 128, (fb + 1) * 128)
                    p_ps = u_ps.tile([128, E], F32, tag="u", name=f"pp{fb}")
                    first = None
                    for ec in range(EB):
                        bi = nc.tensor.matmul(
                            p_ps[:],
                            wv_sb[:, ec, fsl],
                            g_sb[hl][:, ec, :],
                            start=ec == 0,
                            stop=False,
                        )
                        first = first or bi
                    nc.tensor.matmul(
                        p_ps[:],
                        bias_sb[0:1, 2, fsl],
                        swp_sb[0:1, hl, :],
                        start=False,
                        stop=True,
                    )
                    nc.vector.tensor_add(P_sb[:, fb, :], p_ps[:], bph_sb[:])
                    return first

                # first half of P (covers the previous head's exp wait)
                p_first = emit_p_group(0)
                emit_p_group(1)
                gate_early = p_first.ins

                # --- T1 = X @ Wq_h (uses X symmetry: lhsT = X slices) ---
                T1_ps = big_ps.tile([128, EB, E], F32, tag="big")
                t1_first = None
                for m in range(EB):
                    msl = slice(m * 128, (m + 1) * 128)
                    for ec in range(EB):
                        bi = nc.tensor.matmul(
                            T1_ps[:, m, :],
                            X_sb[:, ec, msl],
                            wq_sb[:, ec, :],
                            start=ec == 0,
                            stop=ec == EB - 1,
                        )
                        t1_first = t1_first or bi
                gate_mid = t1_first.ins
                T1_sb = t1_pool.tile([128, EB, E], F32R, tag="t1")
                for m in range(EB):
                    nc.vector.tensor_copy(T1_sb[:, m, :], T1_ps[:, m, :])

                # second half of P + previous head's s: both independent of
                # T1, they cover the T1 psum->sbuf copy before attT
                emit_p_group(2)
                emit_p_group(3)
                P_tiles.append(P_sb)
                emit_pending_s()

                # --- attT = Wk_h^T @ T1 + uvec (x) bq + bk (x) wvec ---
                attT_ps = big_ps.tile([128, EB, E], F32, tag="big")
                for fb in range(EB):
                    fsl = slice(fb * 128, (fb + 1) * 128)
                    for ec in range(EB):
                        nc.tensor.matmul(
                            attT_ps[:, fb, :],
                            wk_sb[:, ec, fsl],
                            T1_sb[:, ec, :],
                            start=ec == 0,
                            stop=False,
                        )
                    nc.tensor.matmul(
                        attT_ps[:, fb, :],
                        hvec_sb[0:1, u, hl, 0, fsl],
                        bias_sb[0:1, 0, :],
                        start=False,
                        stop=False,
                    )
                    nc.tensor.matmul(
                        attT_ps[:, fb, :],
                        bias_sb[0:1, 1, fsl],
                        hvec_sb[0:1, u, hl, 1, :],
                        start=False,
                        stop=True,
                    )

                # --- exp (softmax numerator, transposed layout) ---
                A_sb = a_pool.tile([128, EB, E], F32R, tag="a")
                for fb in range(EB):
                    nc.scalar.activation(
                        out=A_sb[:, fb, :],
                        in_=attT_ps[:, fb, :],
                        func=mybir.ActivationFunctionType.Exp,
                        scale=SCALE,
                    )
                A_tiles.append(A_sb)
                pending_s = (A_sb, R_tiles)
                gate_hist.append(gate_early)

            # --- U_h = A_h^T @ P_h ; out = U0*r0 + U1*r1 ---
            out_tiles = [
                o_pool.tile([128, E], F32, tag="o", name=f"ot{u}_{i}")
                for i in range(EB)
            ]
            for hl in range(2):
                if hl == 1:
                    emit_pending_s()  # s of this unit's second head
                for eb in range(EB):
                    esl = slice(eb * 128, (eb + 1) * 128)
                    u_tile = u_ps.tile([128, E], F32, tag="u")
                    for fc in range(EB):
                        nc.tensor.matmul(
                            u_tile[:],
                            A_tiles[hl][:, fc, esl],
                            P_tiles[hl][:, fc, :],
                            start=fc == 0,
                            stop=fc == EB - 1,
                        )
                    if hl == 0:
                        nc.vector.tensor_scalar_mul(
                            out_tiles[eb][:], u_tile[:], R_tiles[0][:, eb, 0:1]
                        )
                    else:
                        nc.vector.scalar_tensor_tensor(
                            out_tiles[eb][:],
                            u_tile[:],
                            R_tiles[1][:, eb, 0:1],
                            out_tiles[eb][:],
                            op0=mybir.AluOpType.mult,
                            op1=mybir.AluOpType.add,
                        )
                    if hl == 1:
                        nc.sync.dma_start(
                            out=out_d.ap()[u, eb * 128 : (eb + 1) * 128, :],
                            in_=out_tiles[eb][:],
                        )

    nc.compile()
    return nc


def _get_nc():
    global _CACHED_NC
    if _CACHED_NC is None:
        _CACHED_NC = build_nc()
    return _CACHED_NC


def make_in_maps(x, Wq, bq, Wk, bk, Wv, bv, Wp, bp):
    x = np.asarray(x, np.float32)
    Wq, Wk, Wv, Wp = (np.asarray(a, np.float32) for a in (Wq, Wk, Wv, Wp))
    bq, bk, bv, bp = (np.asarray(a, np.float32) for a in (bq, bk, bv, bp))
    wp_arr = np.ascontiguousarray(np.stack([Wp[:N], Wp[N:]]))
    swp = np.ascontiguousarray(np.stack([Wp[:N].sum(0), Wp[N:].sum(0)])[None])
    bph = np.ascontiguousarray(0.5 * bp)
    in_maps = []
    for c in range(8):
        b = c // 2
        rs = [2 * (c % 2), 2 * (c % 2) + 1]
        heads = [[2 * r + hl for hl in range(2)] for r in rs]
        xs = x[b].sum(0)

        def tile_w(Wm, h):
            # (E, E) -> [p, t, e] with row t*128+p on partition p
            return Wm[:, h::H].reshape(EB, 128, E).transpose(1, 0, 2)

        wq_arr = np.ascontiguousarray(
            np.stack([[tile_w(Wq, h) for h in hu] for hu in heads])
        )
        wk_arr = np.ascontiguousarray(
            np.stack([[tile_w(Wk, h) for h in hu] for hu in heads])
        )
        wv_arr = np.ascontiguousarray(
            np.stack([[tile_w(Wv, h) for h in hu] for hu in heads])
        )
        bqkv = np.ascontiguousarray(
            np.stack([[[bq[h::H], bk[h::H], bv[h::H]] for h in hu] for hu in heads])[
                :, :, None
            ]
        )
        hvec = np.ascontiguousarray(
            np.stack(
                [
                    [
                        [
                            Wk[:, h::H].T @ xs,
                            Wq[:, h::H].T @ xs + np.float32(N) * bq[h::H],
                        ]
                        for h in hu
                    ]
                    for hu in heads
                ]
            )[None]
        )
        in_maps.append(
            {
                "xn": np.ascontiguousarray(x[b]),
                "wq": wq_arr,
                "wk": wk_arr,
                "wv": wv_arr,
                "wp": wp_arr,
                "swp": swp,
                "hvec": hvec,
                "bqkv": bqkv,
                "bph": bph,
                "ones": np.ones((128, 2), np.float32),
            }
        )
    return in_maps


def assemble_out(results):
    out = np.empty((B, N, E), np.float32)
    for c in range(8):
        b = c // 2
        for ui in range(2):
            r = 2 * (c % 2) + ui
            out[b, r::4, :] = results[c]["out"][ui]
    return out


def run(inputs, trace=False, **spmd_kwargs):
    """Full pipeline; returns (output, BassKernelResults)."""
    nc = _get_nc()
    in_maps = make_in_maps(**inputs)
    res = run_bass_kernel_spmd(
        nc, in_maps, core_ids=list(range(8)), trace=trace, **spmd_kwargs
    )
    return assemble_out(res.results), res


def kernel(**inputs):
    out, _ = run(inputs)
    return out
